# revision 35
# baseline (speedup 1.0000x reference)
"""Trainium2 Bass kernel for LocalDownsampleFlexAttn (24-head attention with
pooled-KV augmentation), head-parallel across 8 NeuronCores.

Sharding: each core owns 3 of the 24 heads. Per core:
  - QKV projections for its 3 heads (column-sliced Wq/Wk/Wv)
  - KV downsampling (4x4 spatial pooling of the 1024 image tokens -> 64)
  - attention over 1536+64 keys
  - partial output projection (row-sliced Wo); host sums the 8 partials + bo.

Layout strategy (v2):
  - x is transposed + cast to bf16 on the host, so the device needs no PE
    transposes at all and input DMA is halved.
  - all weights are cast to bf16 host-side; output partials are fp16.
  - scores are computed transposed ([key, query]); softmax sums come from
    M=1 ones-matmuls col-packed 3-at-a-time into one PSUM bank
    (memset + start=False has_written accumulation).
  - reciprocal of the sums runs on a [128, 12] tile (DMA-transposed via
    DRAM) instead of a [1, 512] row, which would be 8 cycles/elem on one
    DVE lane.
  - attention is software-pipelined: head h scores/exp interleave with
    head h-1 PV + sum matmuls so the PE fills the ACT-exp latency.
"""

import numpy as np
from contextlib import ExitStack

# ---- problem constants (hardcoded per harness contract) ----
S = 1536          # sequence length
DM = 3072         # model dim
NH = 24           # total heads
HD = 128          # head dim
NCORES = 8
HPC = NH // NCORES   # heads per core = 3
CW = HPC * HD        # per-core slice width = 384
TXT = 512
IMG = 1024        # image tokens (32x32)
F = 4             # pooling factor
PK = (IMG // (F * F))  # pooled keys = 64
KALL = S + PK     # 1600 keys
NKT = DM // 128   # 24 model-dim k-tiles
NTT = S // 128    # 12 token tiles
NIT = IMG // 128  # 8 image-token tiles
NKC = (KALL + 127) // 128   # 13 key tiles (last has 64)
ASCALE = float((1.0 / HD) ** 0.5)

_CACHE = {}


def _build_program(debug_taps=False):
    import concourse.bass as bass
    import concourse.bacc as bacc
    import concourse.tile as tile
    from concourse import mybir
    from concourse.masks import make_identity

    f32 = mybir.dt.float32
    f16 = mybir.dt.float16
    bf16 = mybir.dt.bfloat16
    AF = mybir.ActivationFunctionType
    AX = mybir.AxisListType

    nc = bacc.Bacc(
        "TRN2",
        target_bir_lowering=False,
        debug=False,
        enable_asserts=False,
        num_devices=NCORES,
    )

    xt_d = nc.dram_tensor("xt", [DM, S], bf16, kind="ExternalInput").ap()
    wq_d = nc.dram_tensor("wq", [DM, CW], bf16, kind="ExternalInput").ap()
    wk_d = nc.dram_tensor("wk", [DM, CW], bf16, kind="ExternalInput").ap()
    wv_d = nc.dram_tensor("wv", [DM, CW], bf16, kind="ExternalInput").ap()
    bq_d = nc.dram_tensor("bq", [CW], f32, kind="ExternalInput").ap()
    bk_d = nc.dram_tensor("bk", [CW], f32, kind="ExternalInput").ap()
    bv_d = nc.dram_tensor("bv", [CW], bf16, kind="ExternalInput").ap()
    wo_d = nc.dram_tensor("wo", [CW, DM], bf16, kind="ExternalInput").ap()
    pmat_d = nc.dram_tensor("pmat", [IMG, PK], bf16, kind="ExternalInput").ap()
    wfull_d = nc.dram_tensor("wfull", [IMG], f32, kind="ExternalInput").ap()
    out_d = nc.dram_tensor("out", [S, DM], f16, kind="ExternalOutput").ap()

    taps = {}
    if debug_taps:
        taps["qT"] = nc.dram_tensor("dbg_qT", [128, HPC, S], bf16, kind="ExternalOutput").ap()
        taps["kT"] = nc.dram_tensor("dbg_kT", [128, HPC, NKC * 128], bf16, kind="ExternalOutput").ap()
        taps["vA"] = nc.dram_tensor("dbg_vA", [128, HPC, NKC, HD], bf16, kind="ExternalOutput").ap()
        taps["probsT0"] = nc.dram_tensor("dbg_probsT0", [128, NKC, S], bf16, kind="ExternalOutput").ap()
        taps["sums0"] = nc.dram_tensor("dbg_sums0", [128, 512], f32, kind="ExternalOutput").ap()
        taps["rsb0"] = nc.dram_tensor("dbg_rsb0", [128, S], f32, kind="ExternalOutput").ap()
        taps["attnT"] = nc.dram_tensor("dbg_attnT", [128, HPC, S], bf16, kind="ExternalOutput").ap()

    # engine alternator for PSUM->SBUF copies
    _flip = [0]

    def copy_alt(dst, src):
        _flip[0] ^= 1
        if _flip[0]:
            nc.vector.tensor_copy(dst, src)
        else:
            nc.scalar.copy(dst, src)

    with tile.TileContext(nc) as tc, ExitStack() as ctx:
        persist = ctx.enter_context(tc.tile_pool(name="persist", bufs=1))

        # per-head per-partition biases: b[p, h] = bias[h*128 + p]
        bq_sb = persist.tile([128, HPC], f32)
        bk_sb = persist.tile([128, HPC], f32)
        nc.sync.dma_start(
            out=bq_sb, in_=bass.AP(tensor=bq_d.tensor, offset=0, ap=[[1, 128], [128, HPC]])
        )
        nc.sync.dma_start(
            out=bk_sb, in_=bass.AP(tensor=bk_d.tensor, offset=0, ap=[[1, 128], [128, HPC]])
        )
        # v-bias as a [1, CW] row for the K=1 outer-product trick
        bvrow = persist.tile([1, CW], bf16)
        nc.sync.dma_start(out=bvrow, in_=bv_d[None, :])
        ones_row = persist.tile([1, 128], bf16)
        nc.vector.memset(ones_row, 1.0)
        ones_col = persist.tile([128, 1], bf16)
        nc.vector.memset(ones_col, 1.0)
        ident = persist.tile([128, 128], f32)
        make_identity(nc, ident)

        # pooling matrix tiles [128, 8, 64] bf16
        pm_bf = persist.tile([128, NIT, PK], bf16)
        nc.sync.dma_start(
            out=pm_bf,
            in_=bass.AP(tensor=pmat_d.tensor, offset=0,
                        ap=[[PK, 128], [128 * PK, NIT], [1, PK]]),
        )

        # spatial weights broadcast to all partitions: [128, 1024] f32
        wfull_sb = persist.tile([128, IMG], f32)
        nc.sync.dma_start(
            out=wfull_sb,
            in_=bass.AP(tensor=wfull_d.tensor, offset=0, ap=[[0, 128], [1, IMG]]),
        )

        # persistent activations
        qT = persist.tile([128, HPC, S], bf16)          # q^T per head [d, tok]
        kT = persist.tile([128, HPC, NKC * 128], bf16)  # k_all^T per head [d, key]
        vA = persist.tile([128, HPC, NKC, HD], bf16)    # v_all per head [key, kt, d]
        attnR = persist.tile([128, HPC, S], bf16)       # unnormalized pv
        attnT = persist.tile([128, HPC, S], bf16)       # attn^T [d(by head), tok]

        # ---------------- Phase B: QKV projections ----------------
        es_b = ctx.enter_context(ExitStack())
        pw = es_b.enter_context(tc.tile_pool(name="pw", bufs=1))
        pxt = es_b.enter_context(tc.tile_pool(name="pxt", bufs=1))
        # per-kt weight + x^T tiles so the first matmuls only wait on small
        # DMAs (keeps the PE fed from ~2us and lets HAM warm up early)
        xts = []
        wq_t = []
        wk_t = []
        wv_t = []
        for kt in range(NKT):
            for lst, w_d, nm in ((wq_t, wq_d, "wq"), (wk_t, wk_d, "wk"),
                                 (wv_t, wv_d, "wv")):
                t = pw.tile([128, CW], bf16, name=f"{nm}{kt}", tag=f"{nm}{kt}")
                nc.sync.dma_start(out=t, in_=w_d[kt * 128:(kt + 1) * 128, :])
                lst.append(t)
            xtile = pxt.tile([128, S], bf16, name=f"xt{kt}", tag=f"xt{kt}")
            nc.sync.dma_start(out=xtile, in_=xt_d[kt * 128:(kt + 1) * 128, :])
            xts.append(xtile)

        pBqk = es_b.enter_context(tc.tile_pool(name="pBqk", bufs=2, space="PSUM"))
        pBv = es_b.enter_context(tc.tile_pool(name="pBv", bufs=2, space="PSUM"))
        pKp = es_b.enter_context(tc.tile_pool(name="pKp", bufs=2))

        def qk_copy(h, dst, b_sb, ps):
            nc.scalar.activation(
                dst[:, h, 0:S],
                ps,
                AF.Identity,
                bias=b_sb[:, h:h + 1],
                scale=1.0,
            )

        def pooled_k(h):
            # pooled k columns (kT[:, h, 1536:1600]) via DVE weighted reduce
            tmpw = pKp.tile([128, IMG], f32, tag="tmpw")
            for R in range(8):
                nc.vector.tensor_mul(
                    tmpw[:, R * 128:(R + 1) * 128].rearrange(
                        "p (C i j) -> p C i j", C=8, i=4),
                    kT[:, h, TXT + R * 128:TXT + (R + 1) * 128].rearrange(
                        "p (i C j) -> p C i j", i=4, C=8),
                    wfull_sb[:, R * 128:(R + 1) * 128].rearrange(
                        "p (i C j) -> p C i j", i=4, C=8),
                )
            pooled = pKp.tile([128, PK], f32, tag="pooled")
            nc.vector.reduce_sum(
                pooled,
                tmpw.rearrange("p (rc ij) -> p rc ij", ij=F * F),
                axis=AX.X,
            )
            copy_alt(kT[:, h, S:S + PK], pooled)

        def v_group(tt):
            psv = pBv.tile([128, CW], f32, tag="v", name=f"psv{tt}")
            nc.tensor.matmul(psv, ones_row, bvrow, start=True, stop=False)
            return psv

        def v_step(psv, tt, kt):
            nc.tensor.matmul(
                psv,
                xts[kt][:, tt * 128:(tt + 1) * 128],
                wv_t[kt],
                start=False,
                stop=(kt == NKT - 1),
            )

        def v_copy(psv, tt):
            copy_alt(
                vA[:, :, tt, :],
                psv.rearrange("p (h d) -> p h d", h=HPC),
            )

        # First sweep is DMA-paced: interleave q(h0), k(h0), v(tt0), v(tt1)
        # per arriving kt chunk so PE demand (~1.7us/chunk) matches DMA
        # arrival (~1.6us/chunk) and the PE never starves.
        psq0 = pBqk.tile([128, S], f32, tag="qk", name="psq0")
        psk0 = pBqk.tile([128, S], f32, tag="qk", name="psk0")
        psv0 = v_group(0)
        psv1 = v_group(1)
        for kt in range(NKT):
            for ps, w_t in ((psq0, wq_t), (psk0, wk_t)):
                for c in range(3):
                    nc.tensor.matmul(
                        ps[:, c * 512:(c + 1) * 512],
                        w_t[kt][:, 0:128],
                        xts[kt][:, c * 512:(c + 1) * 512],
                        start=(kt == 0),
                        stop=(kt == NKT - 1),
                    )
            v_step(psv0, 0, kt)
            v_step(psv1, 1, kt)
        qk_copy(0, qT, bq_sb, psq0)
        qk_copy(0, kT, bk_sb, psk0)
        v_copy(psv0, 0)
        v_copy(psv1, 1)
        pooled_k(0)

        # remaining heads + token tiles run from resident SBUF at full rate
        for h in range(1, HPC):
            for w_t, b_sb, dst in ((wq_t, bq_sb, qT), (wk_t, bk_sb, kT)):
                ps = pBqk.tile([128, S], f32, tag="qk")
                for kt in range(NKT):
                    for c in range(3):
                        nc.tensor.matmul(
                            ps[:, c * 512:(c + 1) * 512],
                            w_t[kt][:, h * 128:(h + 1) * 128],
                            xts[kt][:, c * 512:(c + 1) * 512],
                            start=(kt == 0),
                            stop=(kt == NKT - 1),
                        )
                qk_copy(h, dst, b_sb, ps)
            pooled_k(h)

        for tt in range(2, NTT):
            psv = v_group(tt)
            for kt in range(NKT):
                v_step(psv, tt, kt)
            v_copy(psv, tt)

        # weights + x^T + B psum pools no longer needed
        # (pooled v runs inside slot 0 of phase C, off the pCsum pool)
        es_b.close()

        # ---------------- Phase C: attention (software-pipelined) ----------
        # scores computed TRANSPOSED ([key, query]); per c-tile: 3 N=512 MMs
        # sharing one LDWEIGHTS + a single [cs, 1536] exp on ACT. While head
        # h's scores ping-pong with ACT, the PE runs head h-1's PV and sum
        # matmuls, interleaved per c-step so the PE stays dense.
        # PSUM budget: scores 3 + pv 3 + sums 2 banks = 8.
        pDw = ctx.enter_context(tc.tile_pool(name="pDw", bufs=1))
        wo_sb = pDw.tile([128, HPC, DM], bf16)
        nc.sync.dma_start(
            out=wo_sb,
            in_=bass.AP(tensor=wo_d.tensor, offset=0,
                        ap=[[DM, 128], [128 * DM, HPC], [1, DM]]),
        )

        with tc.tile_pool(name="pC", bufs=1) as pC, \
             tc.tile_pool(name="pCT", bufs=1) as pCT, \
             tc.tile_pool(name="pCd", bufs=2, space="DRAM") as pCd, \
             tc.tile_pool(name="pCs", bufs=1, space="PSUM") as pCs, \
             tc.tile_pool(name="pCpv", bufs=3, space="PSUM") as pCpv, \
             tc.tile_pool(name="pCsum", bufs=2, space="PSUM") as pCsum:

            state = {}  # per-head live tiles

            def emit_pooled_v(h):
                # pooled v rows (keys 1536:1600 -> tile 12, rows 0:64)
                psp = pCsum.tile([128, HD], f32, tag="sum", name=f"psp{h}")
                for it in range(NIT):
                    nc.tensor.matmul(
                        psp[:PK, :],
                        pm_bf[:, it, :],
                        vA[:, h, (TXT // 128) + it, :],
                        start=(it == 0),
                        stop=(it == NIT - 1),
                    )
                copy_alt(vA[:PK, h, NKC - 1, :], psp[:PK, :])

            def emit_scores(h, c):
                cs = 128 if c < NKC - 1 else PK
                if c == 0:
                    state[h] = {
                        "probsT": pCT.tile([128, NKC, S], bf16, tag="probsT",
                                           bufs=2, name=f"probsT{h}"),
                    }
                psc = pCs.tile([128, S], f32, tag="s", bufs=1, name=f"psc{h}_{c}")
                for g in range(3):
                    nc.tensor.matmul(
                        psc[:cs, g * 512:(g + 1) * 512],
                        kT[:, h, c * 128:c * 128 + cs],
                        qT[:, h, g * 512:(g + 1) * 512],
                        start=True,
                        stop=True,
                    )
                nc.scalar.activation(
                    state[h]["probsT"][:cs, c, :],
                    psc[:cs, :],
                    AF.Exp,
                    bias=0.0,
                    scale=ASCALE,
                )

            def emit_pv(h, c):
                cs = 128 if c < NKC - 1 else PK
                st = state[h]
                if c == 0:
                    st["ppv"] = [
                        pCpv.tile([128, 512], f32, tag="pv", bufs=3,
                                  name=f"ppv{h}_{g}")
                        for g in range(3)
                    ]
                probsT = st["probsT"]
                for g in range(3):
                    nc.tensor.matmul(
                        st["ppv"][g],
                        vA[:cs, h, c, :],
                        probsT[:cs, c, g * 512:(g + 1) * 512],
                        start=(c == 0),
                        stop=(c == NKC - 1),
                    )

            def emit_sums_step(h, s):
                # s-th of the 39 sum matmuls: g = s // 13, cc = s % 13.
                # g-chains run sequentially through 2 rotating psum banks.
                st = state[h]
                g, cc = divmod(s, NKC)
                cs = 128 if cc < NKC - 1 else PK
                if s == 0:
                    st["srow"] = pC.tile([1, S], f32, tag="srow", bufs=2,
                                         name=f"srow{h}")
                if cc == 0:
                    st["pssum"] = pCsum.tile([1, 512], f32, tag="sum", bufs=2,
                                             name=f"pssum{h}_{g}")
                nc.tensor.matmul(
                    st["pssum"],
                    ones_col[:cs, :],
                    st["probsT"][:cs, cc, g * 512:(g + 1) * 512],
                    start=(cc == 0),
                    stop=(cc == NKC - 1),
                )
                if cc == NKC - 1:
                    copy_alt(st["srow"][0:1, g * 512:(g + 1) * 512], st["pssum"])

            def emit_chain_pre(h):
                # 1/sums: route the [1,1536] q-major row through DRAM into
                # [128,12] so the DVE reciprocal runs on 128 lanes.
                st = state[h]
                rdram = pCd.tile([1, S], f32, tag="rd", bufs=2, name=f"rd{h}")
                nc.sync.dma_start(out=rdram, in_=st["srow"])
                rt = pC.tile([128, NTT], f32, tag="rt", bufs=2, name=f"rt{h}")
                nc.sync.dma_start(
                    out=rt,
                    in_=bass.AP(tensor=rdram.tensor, offset=rdram.offset,
                                ap=[[1, 128], [128, NTT]]),
                )
                rti = pC.tile([128, NTT], f32, tag="rti", bufs=2, name=f"rti{h}")
                nc.vector.reciprocal(rti, rt)
                st["rti"] = rti

            def emit_chain_post(h):
                # PE-transpose [128,12] -> [12,128] so the DRAM image of the
                # reciprocals is q-major and the broadcast read is contiguous.
                # Emitted late so the transpose never blocks pending matmuls
                # while the reciprocal chain is still in flight.
                st = state[h]
                pstr = pCsum.tile([NTT, 128], f32, tag="sum", name=f"pstr{h}")
                nc.tensor.transpose(pstr, st["rti"], ident)
                rtt = pC.tile([NTT, 128], f32, tag="rtt", bufs=2, name=f"rtt{h}")
                nc.vector.tensor_copy(rtt, pstr)
                rdram2 = pCd.tile([NTT, 128], f32, tag="rd2", bufs=2,
                                  name=f"rd2{h}")
                nc.sync.dma_start(out=rdram2, in_=rtt)
                rsb = pC.tile([128, S], f32, tag="rsb", bufs=2, name=f"rsb{h}")
                nc.sync.dma_start(
                    out=rsb,
                    in_=bass.AP(tensor=rdram2.tensor, offset=rdram2.offset,
                                ap=[[0, 128], [1, S]]),
                )
                st["rsb"] = rsb

            def emit_attnR(h):
                st = state[h]
                for g in range(3):
                    copy_alt(attnR[:, h, g * 512:(g + 1) * 512], st["ppv"][g])

            def emit_ttmult(h):
                st = state[h]
                rsb = st["rsb"]
                for g in range(3):
                    nc.vector.tensor_mul(
                        attnT[:, h, g * 512:(g + 1) * 512],
                        attnR[:, h, g * 512:(g + 1) * 512],
                        rsb[:, g * 512:(g + 1) * 512],
                    )

            # pipeline: slot h runs scores(h) + pv/sums(h-1) per c-step.
            # The LAST head's sums run as a block at the end of slot HPC-1
            # (ACT has no more exps there), so its reciprocal chain overlaps
            # the drain slot's pv matmuls instead of being tail-exposed.
            for h in range(HPC + 1):
                for c in range(NKC):
                    if h < HPC:
                        emit_scores(h, c)
                    if h == 0 and c < HPC:
                        emit_pooled_v(c)
                    if h >= 1:
                        emit_pv(h - 1, c)
                        if h - 1 < HPC - 1:
                            for s in (3 * c, 3 * c + 1, 3 * c + 2):
                                emit_sums_step(h - 1, s)
                                if s == 38:
                                    emit_chain_pre(h - 1)
                if h == HPC - 1:
                    for s in range(3 * NKC):
                        emit_sums_step(HPC - 1, s)
                    emit_chain_pre(HPC - 1)
                if h >= 1:
                    emit_attnR(h - 1)
                    emit_chain_post(h - 1)
                    emit_ttmult(h - 1)

            if debug_taps:
                nc.sync.dma_start(out=taps["qT"], in_=qT)
                nc.sync.dma_start(out=taps["kT"], in_=kT)
                nc.sync.dma_start(out=taps["vA"], in_=vA)
                nc.sync.dma_start(out=taps["attnT"], in_=attnT)

        # ---------------- Phase D: output projection (partial) ----------------
        with tc.tile_pool(name="pD", bufs=3) as pD, \
             tc.tile_pool(name="pDpsum", bufs=2, space="PSUM") as pDpsum:
            for qt in range(NTT):
                for g in range(2):
                    pso = pDpsum.tile([128, 1536], f32, tag="o")
                    for kt in range(HPC):
                        for c in range(3):
                            nc.tensor.matmul(
                                pso[:, c * 512:(c + 1) * 512],
                                attnT[:, kt, qt * 128:(qt + 1) * 128],
                                wo_sb[:, kt, g * 1536 + c * 512:g * 1536 + (c + 1) * 512],
                                start=(kt == 0),
                                stop=(kt == HPC - 1),
                            )
                    outsb = pD.tile([128, 1536], f16, tag="outsb")
                    copy_alt(outsb, pso)
                    nc.sync.dma_start(
                        out=out_d[qt * 128:(qt + 1) * 128, g * 1536:(g + 1) * 1536],
                        in_=outsb,
                    )

    nc.compile()
    return nc


def _get_program(debug_taps=False):
    key = ("nc", debug_taps)
    if key not in _CACHE:
        _CACHE[key] = _build_program(debug_taps=debug_taps)
    return _CACHE[key]


def _prep_in_maps(hidden_states, Wq, bq, Wk, bk, Wv, bv, Wo, spatial_weight):
    import ml_dtypes

    bf16 = ml_dtypes.bfloat16
    x = np.asarray(hidden_states, dtype=np.float32).reshape(S, DM)
    xt = np.ascontiguousarray(x.T.astype(bf16))
    Wq = np.asarray(Wq, dtype=np.float32)
    Wk = np.asarray(Wk, dtype=np.float32)
    Wv = np.asarray(Wv, dtype=np.float32)
    Wo = np.asarray(Wo, dtype=np.float32)
    bq = np.asarray(bq, dtype=np.float32)
    bk = np.asarray(bk, dtype=np.float32)
    bv = np.asarray(bv, dtype=np.float32)

    w = np.asarray(spatial_weight, dtype=np.float32).reshape(F, F)  # [i, j]
    # wfull[t] for t = 128R + 32i + 4C + j  -> broadcast w over (R, C)
    wfull = np.ascontiguousarray(
        np.broadcast_to(w[None, :, None, :], (8, F, 8, F)).reshape(IMG)
    )
    # pmat[t, R*8+C] = w[i, j] for t in block (R, C)
    pmat = np.zeros((8, F, 8, F, 8, 8), dtype=np.float32)
    for R in range(8):
        for C in range(8):
            pmat[R, :, C, :, R, C] = w
    pmat = np.ascontiguousarray(pmat.reshape(IMG, PK).astype(bf16))

    in_maps = []
    for c in range(NCORES):
        sl = slice(c * CW, (c + 1) * CW)
        in_maps.append({
            "xt": xt,
            "wq": np.ascontiguousarray(Wq[:, sl].astype(bf16)),
            "wk": np.ascontiguousarray(Wk[:, sl].astype(bf16)),
            "wv": np.ascontiguousarray(Wv[:, sl].astype(bf16)),
            "bq": np.ascontiguousarray(bq[sl]),
            "bk": np.ascontiguousarray(bk[sl]),
            "bv": np.ascontiguousarray(bv[sl].astype(bf16)),
            "wo": np.ascontiguousarray(Wo[sl, :].astype(bf16)),
            "pmat": pmat,
            "wfull": wfull,
        })
    return in_maps


def _run(inputs, trace=False, trace_kwargs=None, debug_taps=False):
    from concourse import bass_utils

    nc = _get_program(debug_taps=debug_taps)
    in_maps = _prep_in_maps(
        inputs["hidden_states"], inputs["Wq"], inputs["bq"], inputs["Wk"],
        inputs["bk"], inputs["Wv"], inputs["bv"], inputs["Wo"],
        inputs["spatial_weight"],
    )
    res = bass_utils.run_bass_kernel_spmd(
        nc, in_maps, list(range(NCORES)), trace=trace,
        **(trace_kwargs or {}),
    )
    partial = np.zeros((S, DM), dtype=np.float32)
    for r in res.results:
        partial += r["out"].astype(np.float32)
    out = partial + np.asarray(inputs["bo"], dtype=np.float32)[None, :]
    return out.reshape(1, S, DM).astype(np.float32), res


def kernel(**inputs):
    h = int(inputs.get("height", 32))
    w = int(inputs.get("width", 32))
    assert h == 32 and w == 32, (h, w)
    out, _ = _run(inputs, trace=False)
    return out


# revision 38
# speedup vs baseline: 1.0334x; 1.0334x over previous
"""Trainium2 Bass kernel for LocalDownsampleFlexAttn (24-head attention with
pooled-KV augmentation), head-parallel across 8 NeuronCores.

Sharding: each core owns 3 of the 24 heads. Per core:
  - QKV projections for its 3 heads (column-sliced Wq/Wk/Wv)
  - KV downsampling (4x4 spatial pooling of the 1024 image tokens -> 64)
  - attention over 1536+64 keys
  - partial output projection (row-sliced Wo); host sums the 8 partials + bo.

Layout strategy (v2):
  - x is transposed + cast to bf16 on the host, so the device needs no PE
    transposes at all and input DMA is halved.
  - all weights are cast to bf16 host-side; output partials are fp16.
  - scores are computed transposed ([key, query]); softmax sums come from
    M=1 ones-matmuls col-packed 3-at-a-time into one PSUM bank
    (memset + start=False has_written accumulation).
  - reciprocal of the sums runs on a [128, 12] tile (DMA-transposed via
    DRAM) instead of a [1, 512] row, which would be 8 cycles/elem on one
    DVE lane.
  - attention is software-pipelined: head h scores/exp interleave with
    head h-1 PV + sum matmuls so the PE fills the ACT-exp latency.
"""

import numpy as np
from contextlib import ExitStack

# ---- problem constants (hardcoded per harness contract) ----
S = 1536          # sequence length
DM = 3072         # model dim
NH = 24           # total heads
HD = 128          # head dim
NCORES = 8
HPC = NH // NCORES   # heads per core = 3
CW = HPC * HD        # per-core slice width = 384
TXT = 512
IMG = 1024        # image tokens (32x32)
F = 4             # pooling factor
PK = (IMG // (F * F))  # pooled keys = 64
KALL = S + PK     # 1600 keys
NKT = DM // 128   # 24 model-dim k-tiles
NTT = S // 128    # 12 token tiles
NIT = IMG // 128  # 8 image-token tiles
NKC = (KALL + 127) // 128   # 13 key tiles (last has 64)
ASCALE = float((1.0 / HD) ** 0.5)

_CACHE = {}


def _build_program(debug_taps=False):
    import concourse.bass as bass
    import concourse.bacc as bacc
    import concourse.tile as tile
    from concourse import mybir
    from concourse.masks import make_identity

    f32 = mybir.dt.float32
    f16 = mybir.dt.float16
    bf16 = mybir.dt.bfloat16
    AF = mybir.ActivationFunctionType
    AX = mybir.AxisListType

    nc = bacc.Bacc(
        "TRN2",
        target_bir_lowering=False,
        debug=False,
        enable_asserts=False,
        num_devices=NCORES,
    )

    xt_d = nc.dram_tensor("xt", [DM, S], bf16, kind="ExternalInput").ap()
    wq_d = nc.dram_tensor("wq", [DM, CW], bf16, kind="ExternalInput").ap()
    wk_d = nc.dram_tensor("wk", [DM, CW], bf16, kind="ExternalInput").ap()
    wv_d = nc.dram_tensor("wv", [DM, CW], bf16, kind="ExternalInput").ap()
    bq_d = nc.dram_tensor("bq", [CW], f32, kind="ExternalInput").ap()
    bk_d = nc.dram_tensor("bk", [CW], f32, kind="ExternalInput").ap()
    bv_d = nc.dram_tensor("bv", [CW], bf16, kind="ExternalInput").ap()
    wo_d = nc.dram_tensor("wo", [CW, DM], bf16, kind="ExternalInput").ap()
    pmat_d = nc.dram_tensor("pmat", [IMG, PK], bf16, kind="ExternalInput").ap()
    wfull_d = nc.dram_tensor("wfull", [IMG], f32, kind="ExternalInput").ap()
    out_d = nc.dram_tensor("out", [S, DM], f16, kind="ExternalOutput").ap()

    taps = {}
    if debug_taps:
        taps["qT"] = nc.dram_tensor("dbg_qT", [128, HPC, S], bf16, kind="ExternalOutput").ap()
        taps["kT"] = nc.dram_tensor("dbg_kT", [128, HPC, NKC * 128], bf16, kind="ExternalOutput").ap()
        taps["vA"] = nc.dram_tensor("dbg_vA", [128, HPC, NKC, HD], bf16, kind="ExternalOutput").ap()
        taps["probsT0"] = nc.dram_tensor("dbg_probsT0", [128, NKC, S], bf16, kind="ExternalOutput").ap()
        taps["sums0"] = nc.dram_tensor("dbg_sums0", [128, 512], f32, kind="ExternalOutput").ap()
        taps["rsb0"] = nc.dram_tensor("dbg_rsb0", [128, S], f32, kind="ExternalOutput").ap()
        taps["attnT"] = nc.dram_tensor("dbg_attnT", [128, HPC, S], bf16, kind="ExternalOutput").ap()

    # engine alternator for PSUM->SBUF copies
    _flip = [0]

    def copy_alt(dst, src):
        _flip[0] ^= 1
        if _flip[0]:
            nc.vector.tensor_copy(dst, src)
        else:
            nc.scalar.copy(dst, src)

    with tile.TileContext(nc) as tc, ExitStack() as ctx:
        persist = ctx.enter_context(tc.tile_pool(name="persist", bufs=1))

        # per-head per-partition biases: b[p, h] = bias[h*128 + p]
        bq_sb = persist.tile([128, HPC], f32)
        bk_sb = persist.tile([128, HPC], f32)
        nc.sync.dma_start(
            out=bq_sb, in_=bass.AP(tensor=bq_d.tensor, offset=0, ap=[[1, 128], [128, HPC]])
        )
        nc.sync.dma_start(
            out=bk_sb, in_=bass.AP(tensor=bk_d.tensor, offset=0, ap=[[1, 128], [128, HPC]])
        )
        # v-bias as a [1, CW] row for the K=1 outer-product trick
        bvrow = persist.tile([1, CW], bf16)
        nc.sync.dma_start(out=bvrow, in_=bv_d[None, :])
        ones_row = persist.tile([1, 128], bf16)
        nc.vector.memset(ones_row, 1.0)
        ones_col = persist.tile([128, 1], bf16)
        nc.vector.memset(ones_col, 1.0)
        ident = persist.tile([128, 128], f32)
        make_identity(nc, ident)

        # pooling matrix tiles [128, 8, 64] bf16
        pm_bf = persist.tile([128, NIT, PK], bf16)
        nc.sync.dma_start(
            out=pm_bf,
            in_=bass.AP(tensor=pmat_d.tensor, offset=0,
                        ap=[[PK, 128], [128 * PK, NIT], [1, PK]]),
        )

        # spatial weights broadcast to all partitions: [128, 1024] f32
        wfull_sb = persist.tile([128, IMG], f32)
        nc.sync.dma_start(
            out=wfull_sb,
            in_=bass.AP(tensor=wfull_d.tensor, offset=0, ap=[[0, 128], [1, IMG]]),
        )

        # persistent activations
        qT = persist.tile([128, HPC, S], bf16)          # q^T per head [d, tok]
        kT = persist.tile([128, HPC, NKC * 128], bf16)  # k_all^T per head [d, key]
        vA = persist.tile([128, HPC, NKC, HD], bf16)    # v_all per head [key, kt, d]
        attnR = persist.tile([128, HPC, S], bf16)       # unnormalized pv
        attnT = persist.tile([128, HPC, S], bf16)       # attn^T [d(by head), tok]

        # ---------------- Phase B: QKV projections ----------------
        es_b = ctx.enter_context(ExitStack())
        pw = es_b.enter_context(tc.tile_pool(name="pw", bufs=1))
        pxt = es_b.enter_context(tc.tile_pool(name="pxt", bufs=1))
        # per-kt weight + x^T tiles so the first matmuls only wait on small
        # DMAs (keeps the PE fed from ~2us and lets HAM warm up early)
        def load_w_tiles(w_d, nm):
            tiles = []
            for kt in range(NKT):
                t = pw.tile([128, CW], bf16, name=f"{nm}{kt}", tag=f"{nm}{kt}")
                nc.sync.dma_start(out=t, in_=w_d[kt * 128:(kt + 1) * 128, :])
                tiles.append(t)
            return tiles

        # q(h0) is the first consumer and needs only wq + x^T — keep the
        # early DMA stream undiluted (wk/wv follow once x^T is in flight)
        xts = []
        wq_t = []
        for kt in range(NKT):
            t = pw.tile([128, CW], bf16, name=f"wq{kt}", tag=f"wq{kt}")
            nc.sync.dma_start(out=t, in_=wq_d[kt * 128:(kt + 1) * 128, :])
            wq_t.append(t)
            xtile = pxt.tile([128, S], bf16, name=f"xt{kt}", tag=f"xt{kt}")
            nc.sync.dma_start(out=xtile, in_=xt_d[kt * 128:(kt + 1) * 128, :])
            xts.append(xtile)
        wk_t = load_w_tiles(wk_d, "wk")
        wv_t = load_w_tiles(wv_d, "wv")

        pBqk = es_b.enter_context(tc.tile_pool(name="pBqk", bufs=2, space="PSUM"))
        pBv = es_b.enter_context(tc.tile_pool(name="pBv", bufs=2, space="PSUM"))
        pKp = es_b.enter_context(tc.tile_pool(name="pKp", bufs=2))

        def qk_copy(h, dst, b_sb, ps):
            nc.scalar.activation(
                dst[:, h, 0:S],
                ps,
                AF.Identity,
                bias=b_sb[:, h:h + 1],
                scale=1.0,
            )

        def pooled_k(h):
            # pooled k columns (kT[:, h, 1536:1600]) via DVE weighted reduce
            tmpw = pKp.tile([128, IMG], f32, tag="tmpw")
            for R in range(8):
                nc.vector.tensor_mul(
                    tmpw[:, R * 128:(R + 1) * 128].rearrange(
                        "p (C i j) -> p C i j", C=8, i=4),
                    kT[:, h, TXT + R * 128:TXT + (R + 1) * 128].rearrange(
                        "p (i C j) -> p C i j", i=4, C=8),
                    wfull_sb[:, R * 128:(R + 1) * 128].rearrange(
                        "p (i C j) -> p C i j", i=4, C=8),
                )
            pooled = pKp.tile([128, PK], f32, tag="pooled")
            nc.vector.reduce_sum(
                pooled,
                tmpw.rearrange("p (rc ij) -> p rc ij", ij=F * F),
                axis=AX.X,
            )
            copy_alt(kT[:, h, S:S + PK], pooled)

        def v_group(tt):
            psv = pBv.tile([128, CW], f32, tag="v", name=f"psv{tt}")
            nc.tensor.matmul(psv, ones_row, bvrow, start=True, stop=False)
            return psv

        def v_step(psv, tt, kt):
            nc.tensor.matmul(
                psv,
                xts[kt][:, tt * 128:(tt + 1) * 128],
                wv_t[kt],
                start=False,
                stop=(kt == NKT - 1),
            )

        def v_copy(psv, tt):
            copy_alt(
                vA[:, :, tt, :],
                psv.rearrange("p (h d) -> p h d", h=HPC),
            )

        for h in range(HPC):
            for w_t, b_sb, dst in ((wq_t, bq_sb, qT), (wk_t, bk_sb, kT)):
                ps = pBqk.tile([128, S], f32, tag="qk")
                for kt in range(NKT):
                    for c in range(3):
                        nc.tensor.matmul(
                            ps[:, c * 512:(c + 1) * 512],
                            w_t[kt][:, h * 128:(h + 1) * 128],
                            xts[kt][:, c * 512:(c + 1) * 512],
                            start=(kt == 0),
                            stop=(kt == NKT - 1),
                        )
                qk_copy(h, dst, b_sb, ps)
            pooled_k(h)

        for tt in range(NTT):
            psv = v_group(tt)
            for kt in range(NKT):
                v_step(psv, tt, kt)
            v_copy(psv, tt)

        # weights + x^T + B psum pools no longer needed
        # (pooled v runs inside slot 0 of phase C, off the pCsum pool)
        es_b.close()

        # ---------------- Phase C: attention (software-pipelined) ----------
        # scores computed TRANSPOSED ([key, query]); per c-tile: 3 N=512 MMs
        # sharing one LDWEIGHTS + a single [cs, 1536] exp on ACT. While head
        # h's scores ping-pong with ACT, the PE runs head h-1's PV and sum
        # matmuls, interleaved per c-step so the PE stays dense.
        # PSUM budget: scores 3 + pv 3 + sums 2 banks = 8.
        pDw = ctx.enter_context(tc.tile_pool(name="pDw", bufs=1))
        wo_sb = pDw.tile([128, HPC, DM], bf16)
        nc.sync.dma_start(
            out=wo_sb,
            in_=bass.AP(tensor=wo_d.tensor, offset=0,
                        ap=[[DM, 128], [128 * DM, HPC], [1, DM]]),
        )

        with tc.tile_pool(name="pC", bufs=1) as pC, \
             tc.tile_pool(name="pCT", bufs=1) as pCT, \
             tc.tile_pool(name="pCd", bufs=2, space="DRAM") as pCd, \
             tc.tile_pool(name="pCs", bufs=1, space="PSUM") as pCs, \
             tc.tile_pool(name="pCpv", bufs=3, space="PSUM") as pCpv, \
             tc.tile_pool(name="pCsum", bufs=2, space="PSUM") as pCsum:

            state = {}  # per-head live tiles

            def emit_pooled_v(h):
                # pooled v rows (keys 1536:1600 -> tile 12, rows 0:64)
                psp = pCsum.tile([128, HD], f32, tag="sum", name=f"psp{h}")
                for it in range(NIT):
                    nc.tensor.matmul(
                        psp[:PK, :],
                        pm_bf[:, it, :],
                        vA[:, h, (TXT // 128) + it, :],
                        start=(it == 0),
                        stop=(it == NIT - 1),
                    )
                copy_alt(vA[:PK, h, NKC - 1, :], psp[:PK, :])

            def emit_scores(h, c):
                cs = 128 if c < NKC - 1 else PK
                if c == 0:
                    state[h] = {
                        "probsT": pCT.tile([128, NKC, S], bf16, tag="probsT",
                                           bufs=2, name=f"probsT{h}"),
                    }
                psc = pCs.tile([128, S], f32, tag="s", bufs=1, name=f"psc{h}_{c}")
                for g in range(3):
                    nc.tensor.matmul(
                        psc[:cs, g * 512:(g + 1) * 512],
                        kT[:, h, c * 128:c * 128 + cs],
                        qT[:, h, g * 512:(g + 1) * 512],
                        start=True,
                        stop=True,
                    )
                nc.scalar.activation(
                    state[h]["probsT"][:cs, c, :],
                    psc[:cs, :],
                    AF.Exp,
                    bias=0.0,
                    scale=ASCALE,
                )

            def emit_pv(h, c):
                cs = 128 if c < NKC - 1 else PK
                st = state[h]
                if c == 0:
                    st["ppv"] = [
                        pCpv.tile([128, 512], f32, tag="pv", bufs=3,
                                  name=f"ppv{h}_{g}")
                        for g in range(3)
                    ]
                probsT = st["probsT"]
                for g in range(3):
                    nc.tensor.matmul(
                        st["ppv"][g],
                        vA[:cs, h, c, :],
                        probsT[:cs, c, g * 512:(g + 1) * 512],
                        start=(c == 0),
                        stop=(c == NKC - 1),
                    )

            def emit_sums_step(h, s):
                # s-th of the 39 sum matmuls: g = s // 13, cc = s % 13.
                # g-chains run sequentially through 2 rotating psum banks.
                st = state[h]
                g, cc = divmod(s, NKC)
                cs = 128 if cc < NKC - 1 else PK
                if s == 0:
                    st["srow"] = pC.tile([1, S], f32, tag="srow", bufs=2,
                                         name=f"srow{h}")
                if cc == 0:
                    st["pssum"] = pCsum.tile([1, 512], f32, tag="sum", bufs=2,
                                             name=f"pssum{h}_{g}")
                nc.tensor.matmul(
                    st["pssum"],
                    ones_col[:cs, :],
                    st["probsT"][:cs, cc, g * 512:(g + 1) * 512],
                    start=(cc == 0),
                    stop=(cc == NKC - 1),
                )
                if cc == NKC - 1:
                    copy_alt(st["srow"][0:1, g * 512:(g + 1) * 512], st["pssum"])

            def emit_chain_pre(h):
                # 1/sums: route the [1,1536] q-major row through DRAM into
                # [128,12] so the DVE reciprocal runs on 128 lanes.
                st = state[h]
                rdram = pCd.tile([1, S], f32, tag="rd", bufs=2, name=f"rd{h}")
                nc.sync.dma_start(out=rdram, in_=st["srow"])
                rt = pC.tile([128, NTT], f32, tag="rt", bufs=2, name=f"rt{h}")
                nc.sync.dma_start(
                    out=rt,
                    in_=bass.AP(tensor=rdram.tensor, offset=rdram.offset,
                                ap=[[1, 128], [128, NTT]]),
                )
                rti = pC.tile([128, NTT], f32, tag="rti", bufs=2, name=f"rti{h}")
                nc.vector.reciprocal(rti, rt)
                st["rti"] = rti

            def emit_chain_post(h):
                # PE-transpose [128,12] -> [12,128] so the DRAM image of the
                # reciprocals is q-major and the broadcast read is contiguous.
                # Emitted late so the transpose never blocks pending matmuls
                # while the reciprocal chain is still in flight.
                st = state[h]
                pstr = pCsum.tile([NTT, 128], f32, tag="sum", name=f"pstr{h}")
                nc.tensor.transpose(pstr, st["rti"], ident)
                rtt = pC.tile([NTT, 128], f32, tag="rtt", bufs=2, name=f"rtt{h}")
                nc.vector.tensor_copy(rtt, pstr)
                rdram2 = pCd.tile([NTT, 128], f32, tag="rd2", bufs=2,
                                  name=f"rd2{h}")
                nc.sync.dma_start(out=rdram2, in_=rtt)
                rsb = pC.tile([128, S], f32, tag="rsb", bufs=2, name=f"rsb{h}")
                nc.sync.dma_start(
                    out=rsb,
                    in_=bass.AP(tensor=rdram2.tensor, offset=rdram2.offset,
                                ap=[[0, 128], [1, S]]),
                )
                st["rsb"] = rsb

            def emit_attnR(h):
                st = state[h]
                for g in range(3):
                    copy_alt(attnR[:, h, g * 512:(g + 1) * 512], st["ppv"][g])

            def emit_ttmult(h):
                st = state[h]
                rsb = st["rsb"]
                for g in range(3):
                    nc.vector.tensor_mul(
                        attnT[:, h, g * 512:(g + 1) * 512],
                        attnR[:, h, g * 512:(g + 1) * 512],
                        rsb[:, g * 512:(g + 1) * 512],
                    )

            # pipeline: slot h runs scores(h) + pv/sums(h-1) per c-step.
            # In the last (drain) slot there are no scores, so run the sums
            # first: the reciprocal DMA chain then overlaps the pv matmuls.
            for h in range(HPC + 1):
                last = (h == HPC)
                if last:
                    for s in range(3 * NKC):
                        emit_sums_step(h - 1, s)
                    emit_chain_pre(h - 1)
                for c in range(NKC):
                    if h < HPC:
                        emit_scores(h, c)
                    if h == 0 and c < HPC:
                        emit_pooled_v(c)
                    if h >= 1:
                        emit_pv(h - 1, c)
                        if not last:
                            for s in (3 * c, 3 * c + 1, 3 * c + 2):
                                emit_sums_step(h - 1, s)
                                if s == 38:
                                    emit_chain_pre(h - 1)
                if h >= 1:
                    emit_attnR(h - 1)
                    emit_chain_post(h - 1)
                    emit_ttmult(h - 1)

            if debug_taps:
                nc.sync.dma_start(out=taps["qT"], in_=qT)
                nc.sync.dma_start(out=taps["kT"], in_=kT)
                nc.sync.dma_start(out=taps["vA"], in_=vA)
                nc.sync.dma_start(out=taps["attnT"], in_=attnT)

        # ---------------- Phase D: output projection (partial) ----------------
        with tc.tile_pool(name="pD", bufs=3) as pD, \
             tc.tile_pool(name="pDpsum", bufs=2, space="PSUM") as pDpsum:
            for qt in range(NTT):
                for g in range(2):
                    pso = pDpsum.tile([128, 1536], f32, tag="o")
                    for kt in range(HPC):
                        for c in range(3):
                            nc.tensor.matmul(
                                pso[:, c * 512:(c + 1) * 512],
                                attnT[:, kt, qt * 128:(qt + 1) * 128],
                                wo_sb[:, kt, g * 1536 + c * 512:g * 1536 + (c + 1) * 512],
                                start=(kt == 0),
                                stop=(kt == HPC - 1),
                            )
                    outsb = pD.tile([128, 1536], f16, tag="outsb")
                    copy_alt(outsb, pso)
                    nc.sync.dma_start(
                        out=out_d[qt * 128:(qt + 1) * 128, g * 1536:(g + 1) * 1536],
                        in_=outsb,
                    )

    nc.compile()
    return nc


def _get_program(debug_taps=False):
    key = ("nc", debug_taps)
    if key not in _CACHE:
        _CACHE[key] = _build_program(debug_taps=debug_taps)
    return _CACHE[key]


def _prep_in_maps(hidden_states, Wq, bq, Wk, bk, Wv, bv, Wo, spatial_weight):
    import ml_dtypes

    bf16 = ml_dtypes.bfloat16
    x = np.asarray(hidden_states, dtype=np.float32).reshape(S, DM)
    xt = np.ascontiguousarray(x.T.astype(bf16))
    Wq = np.asarray(Wq, dtype=np.float32)
    Wk = np.asarray(Wk, dtype=np.float32)
    Wv = np.asarray(Wv, dtype=np.float32)
    Wo = np.asarray(Wo, dtype=np.float32)
    bq = np.asarray(bq, dtype=np.float32)
    bk = np.asarray(bk, dtype=np.float32)
    bv = np.asarray(bv, dtype=np.float32)

    w = np.asarray(spatial_weight, dtype=np.float32).reshape(F, F)  # [i, j]
    # wfull[t] for t = 128R + 32i + 4C + j  -> broadcast w over (R, C)
    wfull = np.ascontiguousarray(
        np.broadcast_to(w[None, :, None, :], (8, F, 8, F)).reshape(IMG)
    )
    # pmat[t, R*8+C] = w[i, j] for t in block (R, C)
    pmat = np.zeros((8, F, 8, F, 8, 8), dtype=np.float32)
    for R in range(8):
        for C in range(8):
            pmat[R, :, C, :, R, C] = w
    pmat = np.ascontiguousarray(pmat.reshape(IMG, PK).astype(bf16))

    in_maps = []
    for c in range(NCORES):
        sl = slice(c * CW, (c + 1) * CW)
        in_maps.append({
            "xt": xt,
            "wq": np.ascontiguousarray(Wq[:, sl].astype(bf16)),
            "wk": np.ascontiguousarray(Wk[:, sl].astype(bf16)),
            "wv": np.ascontiguousarray(Wv[:, sl].astype(bf16)),
            "bq": np.ascontiguousarray(bq[sl]),
            "bk": np.ascontiguousarray(bk[sl]),
            "bv": np.ascontiguousarray(bv[sl].astype(bf16)),
            "wo": np.ascontiguousarray(Wo[sl, :].astype(bf16)),
            "pmat": pmat,
            "wfull": wfull,
        })
    return in_maps


def _run(inputs, trace=False, trace_kwargs=None, debug_taps=False):
    from concourse import bass_utils

    nc = _get_program(debug_taps=debug_taps)
    in_maps = _prep_in_maps(
        inputs["hidden_states"], inputs["Wq"], inputs["bq"], inputs["Wk"],
        inputs["bk"], inputs["Wv"], inputs["bv"], inputs["Wo"],
        inputs["spatial_weight"],
    )
    res = bass_utils.run_bass_kernel_spmd(
        nc, in_maps, list(range(NCORES)), trace=trace,
        **(trace_kwargs or {}),
    )
    partial = np.zeros((S, DM), dtype=np.float32)
    for r in res.results:
        partial += r["out"].astype(np.float32)
    out = partial + np.asarray(inputs["bo"], dtype=np.float32)[None, :]
    return out.reshape(1, S, DM).astype(np.float32), res


def kernel(**inputs):
    h = int(inputs.get("height", 32))
    w = int(inputs.get("width", 32))
    assert h == 32 and w == 32, (h, w)
    out, _ = _run(inputs, trace=False)
    return out


# revision 41
# speedup vs baseline: 1.0479x; 1.0140x over previous
"""Trainium2 Bass kernel for LocalDownsampleFlexAttn (24-head attention with
pooled-KV augmentation), head-parallel across 8 NeuronCores.

Sharding: each core owns 3 of the 24 heads. Per core:
  - QKV projections for its 3 heads (column-sliced Wq/Wk/Wv)
  - KV downsampling (4x4 spatial pooling of the 1024 image tokens -> 64)
  - attention over 1536+64 keys
  - partial output projection (row-sliced Wo); host sums the 8 partials + bo.

Layout strategy (v2):
  - x is transposed + cast to bf16 on the host, so the device needs no PE
    transposes at all and input DMA is halved.
  - all weights are cast to bf16 host-side; output partials are fp16.
  - scores are computed transposed ([key, query]); softmax sums come from
    M=1 ones-matmuls col-packed 3-at-a-time into one PSUM bank
    (memset + start=False has_written accumulation).
  - reciprocal of the sums runs on a [128, 12] tile (DMA-transposed via
    DRAM) instead of a [1, 512] row, which would be 8 cycles/elem on one
    DVE lane.
  - attention is software-pipelined: head h scores/exp interleave with
    head h-1 PV + sum matmuls so the PE fills the ACT-exp latency.
"""

import numpy as np
from contextlib import ExitStack

# ---- problem constants (hardcoded per harness contract) ----
S = 1536          # sequence length
DM = 3072         # model dim
NH = 24           # total heads
HD = 128          # head dim
NCORES = 8
HPC = NH // NCORES   # heads per core = 3
CW = HPC * HD        # per-core slice width = 384
TXT = 512
IMG = 1024        # image tokens (32x32)
F = 4             # pooling factor
PK = (IMG // (F * F))  # pooled keys = 64
KALL = S + PK     # 1600 keys
NKT = DM // 128   # 24 model-dim k-tiles
NTT = S // 128    # 12 token tiles
NIT = IMG // 128  # 8 image-token tiles
NKC = (KALL + 127) // 128   # 13 key tiles (last has 64)
ASCALE = float((1.0 / HD) ** 0.5)

_CACHE = {}


def _build_program(debug_taps=False):
    import concourse.bass as bass
    import concourse.bacc as bacc
    import concourse.tile as tile
    from concourse import mybir
    from concourse.masks import make_identity

    f32 = mybir.dt.float32
    f16 = mybir.dt.float16
    bf16 = mybir.dt.bfloat16
    AF = mybir.ActivationFunctionType
    AX = mybir.AxisListType

    nc = bacc.Bacc(
        "TRN2",
        target_bir_lowering=False,
        debug=False,
        enable_asserts=False,
        num_devices=NCORES,
    )

    xt_d = nc.dram_tensor("xt", [DM, S], bf16, kind="ExternalInput").ap()
    wq_d = nc.dram_tensor("wq", [DM, CW], bf16, kind="ExternalInput").ap()
    wk_d = nc.dram_tensor("wk", [DM, CW], bf16, kind="ExternalInput").ap()
    wv_d = nc.dram_tensor("wv", [DM, CW], bf16, kind="ExternalInput").ap()
    bq_d = nc.dram_tensor("bq", [CW], f32, kind="ExternalInput").ap()
    bk_d = nc.dram_tensor("bk", [CW], f32, kind="ExternalInput").ap()
    bv_d = nc.dram_tensor("bv", [CW], bf16, kind="ExternalInput").ap()
    wo_d = nc.dram_tensor("wo", [CW, DM], bf16, kind="ExternalInput").ap()
    pmat_d = nc.dram_tensor("pmat", [IMG, PK], bf16, kind="ExternalInput").ap()
    wfull_d = nc.dram_tensor("wfull", [IMG], f32, kind="ExternalInput").ap()
    out_d = nc.dram_tensor("out", [S, DM], f16, kind="ExternalOutput").ap()

    taps = {}
    if debug_taps:
        taps["qT"] = nc.dram_tensor("dbg_qT", [128, HPC, S], bf16, kind="ExternalOutput").ap()
        taps["kT"] = nc.dram_tensor("dbg_kT", [128, HPC, NKC * 128], bf16, kind="ExternalOutput").ap()
        taps["vA"] = nc.dram_tensor("dbg_vA", [128, HPC, NKC, HD], bf16, kind="ExternalOutput").ap()
        taps["probsT0"] = nc.dram_tensor("dbg_probsT0", [128, NKC, S], bf16, kind="ExternalOutput").ap()
        taps["sums0"] = nc.dram_tensor("dbg_sums0", [128, 512], f32, kind="ExternalOutput").ap()
        taps["rsb0"] = nc.dram_tensor("dbg_rsb0", [128, S], f32, kind="ExternalOutput").ap()
        taps["attnT"] = nc.dram_tensor("dbg_attnT", [128, HPC, S], bf16, kind="ExternalOutput").ap()

    # engine alternator for PSUM->SBUF copies
    _flip = [0]

    def copy_alt(dst, src):
        _flip[0] ^= 1
        if _flip[0]:
            nc.vector.tensor_copy(dst, src)
        else:
            nc.scalar.copy(dst, src)

    with tile.TileContext(nc) as tc, ExitStack() as ctx:
        persist = ctx.enter_context(tc.tile_pool(name="persist", bufs=1))

        # per-head per-partition biases: b[p, h] = bias[h*128 + p]
        bq_sb = persist.tile([128, HPC], f32)
        bk_sb = persist.tile([128, HPC], f32)
        nc.sync.dma_start(
            out=bq_sb, in_=bass.AP(tensor=bq_d.tensor, offset=0, ap=[[1, 128], [128, HPC]])
        )
        nc.sync.dma_start(
            out=bk_sb, in_=bass.AP(tensor=bk_d.tensor, offset=0, ap=[[1, 128], [128, HPC]])
        )
        # v-bias as a [1, CW] row for the K=1 outer-product trick
        bvrow = persist.tile([1, CW], bf16)
        nc.sync.dma_start(out=bvrow, in_=bv_d[None, :])
        ones_row = persist.tile([1, 128], bf16)
        nc.vector.memset(ones_row, 1.0)
        ones_col = persist.tile([128, 1], bf16)
        nc.vector.memset(ones_col, 1.0)
        ident = persist.tile([128, 128], f32)
        make_identity(nc, ident)

        # pooling matrix tiles [128, 8, 64] bf16
        pm_bf = persist.tile([128, NIT, PK], bf16)
        nc.sync.dma_start(
            out=pm_bf,
            in_=bass.AP(tensor=pmat_d.tensor, offset=0,
                        ap=[[PK, 128], [128 * PK, NIT], [1, PK]]),
        )

        # spatial weights broadcast to all partitions: [128, 1024] f32
        wfull_sb = persist.tile([128, IMG], f32)
        nc.sync.dma_start(
            out=wfull_sb,
            in_=bass.AP(tensor=wfull_d.tensor, offset=0, ap=[[0, 128], [1, IMG]]),
        )

        # persistent activations
        qT = persist.tile([128, HPC, S], bf16)          # q^T per head [d, tok]
        kT = persist.tile([128, HPC, NKC * 128], bf16)  # k_all^T per head [d, key]
        vA = persist.tile([128, HPC, NKC, HD], bf16)    # v_all per head [key, kt, d]
        attnR = persist.tile([128, HPC, S], bf16)       # unnormalized pv
        attnT = persist.tile([128, HPC, S], bf16)       # attn^T [d(by head), tok]

        # ---------------- Phase B: QKV projections ----------------
        es_b = ctx.enter_context(ExitStack())
        pw = es_b.enter_context(tc.tile_pool(name="pw", bufs=1))
        pxt = es_b.enter_context(tc.tile_pool(name="pxt", bufs=1))
        # per-kt weight + x^T tiles so the first matmuls only wait on small
        # DMAs (keeps the PE fed from ~2us and lets HAM warm up early)
        # weights load in 4-kt groups (384KB per DMA — small 96KB transfers
        # run at ~35% DMA efficiency); wchunk(kt) views stay [128, 128]
        WG = 4

        def load_w_groups(w_d, nm):
            groups = []
            for g in range(NKT // WG):
                t = pw.tile([128, WG, CW], bf16, name=f"{nm}{g}", tag=f"{nm}{g}")
                nc.sync.dma_start(
                    out=t,
                    in_=bass.AP(tensor=w_d.tensor, offset=g * WG * 128 * CW,
                                ap=[[CW, 128], [128 * CW, WG], [1, CW]]),
                )
                groups.append(t)
            return groups

        def wchunk(groups, kt, lo, hi):
            return groups[kt // WG][:, kt % WG, lo:hi]

        # q(h0) is the first consumer and needs only wq + x^T — keep the
        # early DMA stream undiluted (wk/wv follow once x^T is in flight)
        xts = []
        wq_t = []
        for kt in range(NKT):
            if kt % WG == 0:
                g = kt // WG
                t = pw.tile([128, WG, CW], bf16, name=f"wq{g}", tag=f"wq{g}")
                nc.sync.dma_start(
                    out=t,
                    in_=bass.AP(tensor=wq_d.tensor, offset=g * WG * 128 * CW,
                                ap=[[CW, 128], [128 * CW, WG], [1, CW]]),
                )
                wq_t.append(t)
            xtile = pxt.tile([128, S], bf16, name=f"xt{kt}", tag=f"xt{kt}")
            nc.sync.dma_start(out=xtile, in_=xt_d[kt * 128:(kt + 1) * 128, :])
            xts.append(xtile)
        wk_t = load_w_groups(wk_d, "wk")
        wv_t = load_w_groups(wv_d, "wv")

        pBqk = es_b.enter_context(tc.tile_pool(name="pBqk", bufs=2, space="PSUM"))
        pBv = es_b.enter_context(tc.tile_pool(name="pBv", bufs=2, space="PSUM"))
        pKp = es_b.enter_context(tc.tile_pool(name="pKp", bufs=2))

        def qk_copy(h, dst, b_sb, ps):
            nc.scalar.activation(
                dst[:, h, 0:S],
                ps,
                AF.Identity,
                bias=b_sb[:, h:h + 1],
                scale=1.0,
            )

        def pooled_k(h):
            # pooled k columns (kT[:, h, 1536:1600]) via DVE weighted reduce
            tmpw = pKp.tile([128, IMG], f32, tag="tmpw")
            for R in range(8):
                nc.vector.tensor_mul(
                    tmpw[:, R * 128:(R + 1) * 128].rearrange(
                        "p (C i j) -> p C i j", C=8, i=4),
                    kT[:, h, TXT + R * 128:TXT + (R + 1) * 128].rearrange(
                        "p (i C j) -> p C i j", i=4, C=8),
                    wfull_sb[:, R * 128:(R + 1) * 128].rearrange(
                        "p (i C j) -> p C i j", i=4, C=8),
                )
            pooled = pKp.tile([128, PK], f32, tag="pooled")
            nc.vector.reduce_sum(
                pooled,
                tmpw.rearrange("p (rc ij) -> p rc ij", ij=F * F),
                axis=AX.X,
            )
            copy_alt(kT[:, h, S:S + PK], pooled)

        def v_group(tt):
            psv = pBv.tile([128, CW], f32, tag="v", name=f"psv{tt}")
            nc.tensor.matmul(psv, ones_row, bvrow, start=True, stop=False)
            return psv

        def v_step(psv, tt, kt):
            nc.tensor.matmul(
                psv,
                xts[kt][:, tt * 128:(tt + 1) * 128],
                wchunk(wv_t, kt, 0, CW),
                start=False,
                stop=(kt == NKT - 1),
            )

        def v_copy(psv, tt):
            copy_alt(
                vA[:, :, tt, :],
                psv.rearrange("p (h d) -> p h d", h=HPC),
            )

        for h in range(HPC):
            for w_t, b_sb, dst in ((wq_t, bq_sb, qT), (wk_t, bk_sb, kT)):
                ps = pBqk.tile([128, S], f32, tag="qk")
                for kt in range(NKT):
                    for c in range(3):
                        nc.tensor.matmul(
                            ps[:, c * 512:(c + 1) * 512],
                            wchunk(w_t, kt, h * 128, (h + 1) * 128),
                            xts[kt][:, c * 512:(c + 1) * 512],
                            start=(kt == 0),
                            stop=(kt == NKT - 1),
                        )
                qk_copy(h, dst, b_sb, ps)
            pooled_k(h)

        for tt in range(NTT):
            psv = v_group(tt)
            for kt in range(NKT):
                v_step(psv, tt, kt)
            v_copy(psv, tt)

        # weights + x^T + B psum pools no longer needed
        # (pooled v runs inside slot 0 of phase C, off the pCsum pool)
        es_b.close()

        # ---------------- Phase C: attention (software-pipelined) ----------
        # scores computed TRANSPOSED ([key, query]); per c-tile: 3 N=512 MMs
        # sharing one LDWEIGHTS + a single [cs, 1536] exp on ACT. While head
        # h's scores ping-pong with ACT, the PE runs head h-1's PV and sum
        # matmuls, interleaved per c-step so the PE stays dense.
        # PSUM budget: scores 3 + pv 3 + sums 2 banks = 8.
        pDw = ctx.enter_context(tc.tile_pool(name="pDw", bufs=1))
        wo_sb = pDw.tile([128, HPC, DM], bf16)
        nc.sync.dma_start(
            out=wo_sb,
            in_=bass.AP(tensor=wo_d.tensor, offset=0,
                        ap=[[DM, 128], [128 * DM, HPC], [1, DM]]),
        )

        with tc.tile_pool(name="pC", bufs=1) as pC, \
             tc.tile_pool(name="pCT", bufs=1) as pCT, \
             tc.tile_pool(name="pCd", bufs=2, space="DRAM") as pCd, \
             tc.tile_pool(name="pCs", bufs=1, space="PSUM") as pCs, \
             tc.tile_pool(name="pCpv", bufs=3, space="PSUM") as pCpv, \
             tc.tile_pool(name="pCsum", bufs=2, space="PSUM") as pCsum:

            state = {}  # per-head live tiles

            def emit_pooled_v(h):
                # pooled v rows (keys 1536:1600 -> tile 12, rows 0:64)
                psp = pCsum.tile([128, HD], f32, tag="sum", name=f"psp{h}")
                for it in range(NIT):
                    nc.tensor.matmul(
                        psp[:PK, :],
                        pm_bf[:, it, :],
                        vA[:, h, (TXT // 128) + it, :],
                        start=(it == 0),
                        stop=(it == NIT - 1),
                    )
                copy_alt(vA[:PK, h, NKC - 1, :], psp[:PK, :])

            def emit_scores(h, c):
                cs = 128 if c < NKC - 1 else PK
                if c == 0:
                    state[h] = {
                        "probsT": pCT.tile([128, NKC, S], bf16, tag="probsT",
                                           bufs=2, name=f"probsT{h}"),
                    }
                psc = pCs.tile([128, S], f32, tag="s", bufs=1, name=f"psc{h}_{c}")
                for g in range(3):
                    nc.tensor.matmul(
                        psc[:cs, g * 512:(g + 1) * 512],
                        kT[:, h, c * 128:c * 128 + cs],
                        qT[:, h, g * 512:(g + 1) * 512],
                        start=True,
                        stop=True,
                    )
                nc.scalar.activation(
                    state[h]["probsT"][:cs, c, :],
                    psc[:cs, :],
                    AF.Exp,
                    bias=0.0,
                    scale=ASCALE,
                )

            def emit_pv(h, c):
                cs = 128 if c < NKC - 1 else PK
                st = state[h]
                if c == 0:
                    st["ppv"] = [
                        pCpv.tile([128, 512], f32, tag="pv", bufs=3,
                                  name=f"ppv{h}_{g}")
                        for g in range(3)
                    ]
                probsT = st["probsT"]
                for g in range(3):
                    nc.tensor.matmul(
                        st["ppv"][g],
                        vA[:cs, h, c, :],
                        probsT[:cs, c, g * 512:(g + 1) * 512],
                        start=(c == 0),
                        stop=(c == NKC - 1),
                    )

            def emit_sums_step(h, s):
                # s-th of the 39 sum matmuls: g = s // 13, cc = s % 13.
                # g-chains run sequentially through 2 rotating psum banks.
                st = state[h]
                g, cc = divmod(s, NKC)
                cs = 128 if cc < NKC - 1 else PK
                if s == 0:
                    st["srow"] = pC.tile([1, S], f32, tag="srow", bufs=2,
                                         name=f"srow{h}")
                if cc == 0:
                    st["pssum"] = pCsum.tile([1, 512], f32, tag="sum", bufs=2,
                                             name=f"pssum{h}_{g}")
                nc.tensor.matmul(
                    st["pssum"],
                    ones_col[:cs, :],
                    st["probsT"][:cs, cc, g * 512:(g + 1) * 512],
                    start=(cc == 0),
                    stop=(cc == NKC - 1),
                )
                if cc == NKC - 1:
                    copy_alt(st["srow"][0:1, g * 512:(g + 1) * 512], st["pssum"])

            def emit_chain_pre(h):
                # 1/sums: route the [1,1536] q-major row through DRAM into
                # [128,12] so the DVE reciprocal runs on 128 lanes.
                st = state[h]
                rdram = pCd.tile([1, S], f32, tag="rd", bufs=2, name=f"rd{h}")
                nc.sync.dma_start(out=rdram, in_=st["srow"])
                rt = pC.tile([128, NTT], f32, tag="rt", bufs=2, name=f"rt{h}")
                nc.sync.dma_start(
                    out=rt,
                    in_=bass.AP(tensor=rdram.tensor, offset=rdram.offset,
                                ap=[[1, 128], [128, NTT]]),
                )
                rti = pC.tile([128, NTT], f32, tag="rti", bufs=2, name=f"rti{h}")
                nc.vector.reciprocal(rti, rt)
                st["rti"] = rti

            def emit_chain_post(h):
                # PE-transpose [128,12] -> [12,128] so the DRAM image of the
                # reciprocals is q-major and the broadcast read is contiguous.
                # Emitted late so the transpose never blocks pending matmuls
                # while the reciprocal chain is still in flight.
                st = state[h]
                pstr = pCsum.tile([NTT, 128], f32, tag="sum", name=f"pstr{h}")
                nc.tensor.transpose(pstr, st["rti"], ident)
                rtt = pC.tile([NTT, 128], f32, tag="rtt", bufs=2, name=f"rtt{h}")
                nc.vector.tensor_copy(rtt, pstr)
                rdram2 = pCd.tile([NTT, 128], f32, tag="rd2", bufs=2,
                                  name=f"rd2{h}")
                nc.sync.dma_start(out=rdram2, in_=rtt)
                rsb = pC.tile([128, S], f32, tag="rsb", bufs=2, name=f"rsb{h}")
                nc.sync.dma_start(
                    out=rsb,
                    in_=bass.AP(tensor=rdram2.tensor, offset=rdram2.offset,
                                ap=[[0, 128], [1, S]]),
                )
                st["rsb"] = rsb

            def emit_attnR(h):
                st = state[h]
                for g in range(3):
                    copy_alt(attnR[:, h, g * 512:(g + 1) * 512], st["ppv"][g])

            def emit_ttmult(h):
                st = state[h]
                rsb = st["rsb"]
                for g in range(3):
                    nc.vector.tensor_mul(
                        attnT[:, h, g * 512:(g + 1) * 512],
                        attnR[:, h, g * 512:(g + 1) * 512],
                        rsb[:, g * 512:(g + 1) * 512],
                    )

            # pipeline: slot h runs scores(h) + pv/sums(h-1) per c-step.
            # In the last (drain) slot there are no scores, so run the sums
            # first: the reciprocal DMA chain then overlaps the pv matmuls.
            for h in range(HPC + 1):
                last = (h == HPC)
                if last:
                    for s in range(3 * NKC):
                        emit_sums_step(h - 1, s)
                    emit_chain_pre(h - 1)
                for c in range(NKC):
                    if h < HPC:
                        emit_scores(h, c)
                    if h == 0 and c < HPC:
                        emit_pooled_v(c)
                    if h >= 1:
                        emit_pv(h - 1, c)
                        if not last:
                            for s in (3 * c, 3 * c + 1, 3 * c + 2):
                                emit_sums_step(h - 1, s)
                                if s == 38:
                                    emit_chain_pre(h - 1)
                if h >= 1:
                    emit_attnR(h - 1)
                    emit_chain_post(h - 1)
                    emit_ttmult(h - 1)

            if debug_taps:
                nc.sync.dma_start(out=taps["qT"], in_=qT)
                nc.sync.dma_start(out=taps["kT"], in_=kT)
                nc.sync.dma_start(out=taps["vA"], in_=vA)
                nc.sync.dma_start(out=taps["attnT"], in_=attnT)

        # ---------------- Phase D: output projection (partial) ----------------
        with tc.tile_pool(name="pD", bufs=3) as pD, \
             tc.tile_pool(name="pDpsum", bufs=2, space="PSUM") as pDpsum:
            for qt in range(NTT):
                for g in range(2):
                    pso = pDpsum.tile([128, 1536], f32, tag="o")
                    for kt in range(HPC):
                        for c in range(3):
                            nc.tensor.matmul(
                                pso[:, c * 512:(c + 1) * 512],
                                attnT[:, kt, qt * 128:(qt + 1) * 128],
                                wo_sb[:, kt, g * 1536 + c * 512:g * 1536 + (c + 1) * 512],
                                start=(kt == 0),
                                stop=(kt == HPC - 1),
                            )
                    outsb = pD.tile([128, 1536], f16, tag="outsb")
                    copy_alt(outsb, pso)
                    nc.sync.dma_start(
                        out=out_d[qt * 128:(qt + 1) * 128, g * 1536:(g + 1) * 1536],
                        in_=outsb,
                    )

    nc.compile()
    return nc


def _get_program(debug_taps=False):
    key = ("nc", debug_taps)
    if key not in _CACHE:
        _CACHE[key] = _build_program(debug_taps=debug_taps)
    return _CACHE[key]


def _prep_in_maps(hidden_states, Wq, bq, Wk, bk, Wv, bv, Wo, spatial_weight):
    import ml_dtypes

    bf16 = ml_dtypes.bfloat16
    x = np.asarray(hidden_states, dtype=np.float32).reshape(S, DM)
    xt = np.ascontiguousarray(x.T.astype(bf16))
    Wq = np.asarray(Wq, dtype=np.float32)
    Wk = np.asarray(Wk, dtype=np.float32)
    Wv = np.asarray(Wv, dtype=np.float32)
    Wo = np.asarray(Wo, dtype=np.float32)
    bq = np.asarray(bq, dtype=np.float32)
    bk = np.asarray(bk, dtype=np.float32)
    bv = np.asarray(bv, dtype=np.float32)

    w = np.asarray(spatial_weight, dtype=np.float32).reshape(F, F)  # [i, j]
    # wfull[t] for t = 128R + 32i + 4C + j  -> broadcast w over (R, C)
    wfull = np.ascontiguousarray(
        np.broadcast_to(w[None, :, None, :], (8, F, 8, F)).reshape(IMG)
    )
    # pmat[t, R*8+C] = w[i, j] for t in block (R, C)
    pmat = np.zeros((8, F, 8, F, 8, 8), dtype=np.float32)
    for R in range(8):
        for C in range(8):
            pmat[R, :, C, :, R, C] = w
    pmat = np.ascontiguousarray(pmat.reshape(IMG, PK).astype(bf16))

    in_maps = []
    for c in range(NCORES):
        sl = slice(c * CW, (c + 1) * CW)
        in_maps.append({
            "xt": xt,
            "wq": np.ascontiguousarray(Wq[:, sl].astype(bf16)),
            "wk": np.ascontiguousarray(Wk[:, sl].astype(bf16)),
            "wv": np.ascontiguousarray(Wv[:, sl].astype(bf16)),
            "bq": np.ascontiguousarray(bq[sl]),
            "bk": np.ascontiguousarray(bk[sl]),
            "bv": np.ascontiguousarray(bv[sl].astype(bf16)),
            "wo": np.ascontiguousarray(Wo[sl, :].astype(bf16)),
            "pmat": pmat,
            "wfull": wfull,
        })
    return in_maps


def _run(inputs, trace=False, trace_kwargs=None, debug_taps=False):
    from concourse import bass_utils

    nc = _get_program(debug_taps=debug_taps)
    in_maps = _prep_in_maps(
        inputs["hidden_states"], inputs["Wq"], inputs["bq"], inputs["Wk"],
        inputs["bk"], inputs["Wv"], inputs["bv"], inputs["Wo"],
        inputs["spatial_weight"],
    )
    res = bass_utils.run_bass_kernel_spmd(
        nc, in_maps, list(range(NCORES)), trace=trace,
        **(trace_kwargs or {}),
    )
    partial = np.zeros((S, DM), dtype=np.float32)
    for r in res.results:
        partial += r["out"].astype(np.float32)
    out = partial + np.asarray(inputs["bo"], dtype=np.float32)[None, :]
    return out.reshape(1, S, DM).astype(np.float32), res


def kernel(**inputs):
    h = int(inputs.get("height", 32))
    w = int(inputs.get("width", 32))
    assert h == 32 and w == 32, (h, w)
    out, _ = _run(inputs, trace=False)
    return out
